# revision 7
# baseline (speedup 1.0000x reference)
"""Davies-Bouldin index (segment_reduce) Trainium2 kernel.

Strategy (one pass over the data instead of the reference's two):
  segsum(|x - A_c|^2)[k] = Q_k - 2*A_k.S_k + n_k*|A_k|^2
with S_k = segsum(x), Q_k = segsum(|x|^2), n_k = counts. The device computes
only S and the per-dim square sums S2 (Q = S2.sum(-1)) via a one-hot matmul
over bf16 data, data-parallel over 8 NeuronCores; counts and all K-sized
math run on the host in fp64.

Device per 128-point block b of a supertile:
  H_b[p,k]  = (cls[p,b] == k)        (DVE tensor_scalar is_equal vs iota row)
  psum     += H_b^T @ [X_b | X_b^2]  (PE, fp32 PSUM accumulation)
X^2 is computed by the ACT engine (Square). Per-core output is [128, 128]
fp32: cols 0:64 = S rows, cols 64:128 = S2 rows; rows 100..127 are padding.
"""

from contextlib import ExitStack

import numpy as np
import ml_dtypes

# ---- hardcoded problem geometry (nn_DBI_44985487458968) ----
N_TOTAL = 2_000_000
D = 64
K = 100
N_CORES = 8
P = 128
KPAD = 128            # one-hot width padded so FWL (128-col weights) kicks in
B = 32                # 128-point blocks per supertile
SUP = P * B           # 4096 points per supertile
PER_CORE = N_TOTAL // N_CORES          # 250_000
NSUP = -(-PER_CORE // SUP)             # 62 supertiles
PADN = NSUP * SUP                      # 253_952 padded points per core
PAD_CLS = 127          # pad points land in ignored one-hot column 127

BF16 = ml_dtypes.bfloat16


def _split_excess_waits(nc):
    """Walrus allows one semaphore wait per instruction (two on
    EventSemaphore). Tile's tail drain aggregates one wait per live proc,
    which this compiler build rejects — hoist the extras into standalone
    NoOp wait-carriers executed just before, same engine, same semantics."""
    import concourse.mybir as mybir

    for bb in nc.main_func.blocks:
        new = []
        for inst in bb.instructions:
            si = inst.sync_info
            limit = 2 if isinstance(inst, mybir.InstEventSemaphore) else 1
            if si is not None and si.on_wait and len(si.on_wait) > limit:
                waits = list(si.on_wait)
                for w in waits[:-limit]:
                    nop = mybir.InstNoOp(
                        name=nc.get_next_instruction_name(),
                        engine=inst.engine,
                        ins=[], outs=[],
                        sync_info=mybir.SyncInfo(on_wait=[w], on_update=[]),
                    )
                    nc.register_instruction(nop)
                    new.append(nop)
                inst.sync_info = mybir.SyncInfo(
                    on_wait=waits[-limit:], on_update=list(si.on_update))
            new.append(inst)
        bb.instructions[:] = new


def _build_module(nsup: int, b: int):
    import concourse.bass as bass
    import concourse.mybir as mybir
    import concourse.tile as tile

    sup_cols = b * D                      # X columns per supertile
    nc = bass.Bass()
    x_in = nc.dram_tensor("x", [nsup, P, sup_cols], mybir.dt.bfloat16,
                          kind="ExternalInput")
    cls_in = nc.dram_tensor("cls", [P, nsup * b], mybir.dt.float32,
                            kind="ExternalInput")
    iota_in = nc.dram_tensor("iota", [P, KPAD], mybir.dt.bfloat16,
                             kind="ExternalInput")
    out = nc.dram_tensor("out", [KPAD, 2 * D], mybir.dt.float32,
                         kind="ExternalOutput")

    n_mm_total = nsup * b
    with ExitStack() as ctx:
        tc = ctx.enter_context(tile.TileContext(nc))
        cpool = ctx.enter_context(tc.tile_pool(name="const", bufs=1))
        xpool = ctx.enter_context(tc.tile_pool(name="x", bufs=3))
        hpool = ctx.enter_context(tc.tile_pool(name="h", bufs=3))
        ppool = ctx.enter_context(tc.tile_pool(name="psum", bufs=1, space="PSUM"))
        opool = ctx.enter_context(tc.tile_pool(name="o", bufs=1))

        iota_t = cpool.tile([P, KPAD], mybir.dt.bfloat16)
        nc.sync.dma_start(out=iota_t[:], in_=iota_in[:])
        cls_t = cpool.tile([P, nsup * b], mybir.dt.float32)
        nc.sync.dma_start(out=cls_t[:], in_=cls_in[:])
        # The TensorScalarPtr ISA format has room for only one semaphore
        # wait; absorb the two const-DMA waits into plain copies so the
        # first one-hot compare below never needs both.
        scratch = cpool.tile([P, 2], mybir.dt.float32)
        nc.vector.tensor_copy(out=scratch[:, 0:1], in_=cls_t[:, 0:1])
        nc.vector.tensor_copy(out=scratch[:, 1:2], in_=iota_t[:, 0:1])

        psum_s = ppool.tile([KPAD, D], mybir.dt.float32)
        psum_q = ppool.tile([KPAD, D], mybir.dt.float32, tag="psq")

        n_mm = 0
        for s in range(nsup):
            xb = xpool.tile([P, 2 * sup_cols], mybir.dt.bfloat16)
            nc.sync.dma_start(out=xb[:, 0:sup_cols], in_=x_in[s])
            nc.scalar.activation(
                out=xb[:, sup_cols:2 * sup_cols],
                in_=xb[:, 0:sup_cols],
                func=mybir.ActivationFunctionType.Square,
            )
            ht = hpool.tile([P, b * KPAD], mybir.dt.bfloat16)
            for j in range(b):
                nc.vector.tensor_scalar(
                    out=ht[:, j * KPAD:(j + 1) * KPAD],
                    in0=iota_t[:],
                    scalar1=cls_t[:, s * b + j:s * b + j + 1],
                    scalar2=None,
                    op0=mybir.AluOpType.is_equal,
                )
            # Two matmuls per block sharing the same stationary H so each
            # rhs has a single producer engine (DMA for X, ACT for X^2) —
            # the MM ISA format has limited semaphore-wait slots.
            for j in range(b):
                nc.tensor.matmul(
                    psum_s[:],
                    lhsT=ht[:, j * KPAD:(j + 1) * KPAD],
                    rhs=xb[:, j * D:(j + 1) * D],
                    start=(n_mm == 0),
                    stop=(n_mm == n_mm_total - 1),
                )
                nc.tensor.matmul(
                    psum_q[:],
                    lhsT=ht[:, j * KPAD:(j + 1) * KPAD],
                    rhs=xb[:, sup_cols + j * D:sup_cols + (j + 1) * D],
                    start=(n_mm == 0),
                    stop=(n_mm == n_mm_total - 1),
                )
                n_mm += 1

        out_sb = opool.tile([KPAD, 2 * D], mybir.dt.float32)
        nc.vector.tensor_copy(out=out_sb[:, 0:D], in_=psum_s[:])
        nc.vector.tensor_copy(out=out_sb[:, D:2 * D], in_=psum_q[:])
        nc.sync.dma_start(out=out[:], in_=out_sb[:])
    _split_excess_waits(nc)
    return nc


def _prep_core_inputs(x_shard: np.ndarray, cls_shard: np.ndarray,
                      nsup: int, b: int) -> dict:
    """Pad + lay out one core's shard for the device kernel."""
    sup = P * b
    padn = nsup * sup
    npts = x_shard.shape[0]
    xb16 = np.zeros((padn, D), dtype=BF16)
    xb16[:npts] = x_shard.astype(BF16)
    clsf = np.full((padn,), PAD_CLS, dtype=np.float32)
    clsf[:npts] = cls_shard.astype(BF16)
    # xb[s, p, j*D+d] = x[s*sup + p*b + j, d]
    x_dev = np.ascontiguousarray(xb16.reshape(nsup, P, b * D))
    # cls_t[p, s*b + j] = cls[s*sup + p*b + j]
    cls_dev = np.ascontiguousarray(
        clsf.reshape(nsup, P, b).transpose(1, 0, 2).reshape(P, nsup * b))
    iota = np.ascontiguousarray(
        np.broadcast_to(np.arange(KPAD, dtype=BF16), (P, KPAD)))
    return {"x": x_dev, "cls": cls_dev, "iota": iota}


def _dbi_from_stats(S: np.ndarray, S2: np.ndarray, n: np.ndarray) -> np.float32:
    S = S.astype(np.float64)
    Q = S2.astype(np.float64).sum(-1)
    n = n.astype(np.float64)
    counts = 1.0 + n
    A = (0.001 + S) / counts[:, None]
    segsq = Q - 2.0 * (A * S).sum(-1) + n * (A * A).sum(-1)
    Si = np.sqrt((0.001 + segsq) / counts)
    diff = A[:, None, :] - A[None, :, :]
    sumsq = (diff * diff).sum(-1)
    eye = np.eye(K, dtype=bool)
    Mij = np.sqrt(np.where(eye, 1.0, sumsq))
    Rij = np.where(eye, 0.0, (Si[:, None] + Si[None, :]) / Mij)
    return np.float32(Rij.max(axis=1).sum() / K)


def kernel(data_points: np.ndarray, clustering: np.ndarray) -> np.ndarray:
    from concourse.bass_utils import run_bass_kernel_spmd

    x = np.asarray(data_points)
    cls = np.asarray(clustering)
    assert x.shape == (N_TOTAL, D), x.shape

    nc = _build_module(NSUP, B)
    in_maps = []
    for c in range(N_CORES):
        sl = slice(c * PER_CORE, (c + 1) * PER_CORE)
        in_maps.append(_prep_core_inputs(x[sl], cls[sl], NSUP, B))
    res = run_bass_kernel_spmd(nc, in_maps, core_ids=list(range(N_CORES)))

    S = np.zeros((K, D), np.float64)
    S2 = np.zeros((K, D), np.float64)
    for r in res.results:
        o = r["out"].astype(np.float64)
        S += o[:K, :D]
        S2 += o[:K, D:]
    n = np.bincount(cls.astype(np.int64), minlength=K).astype(np.float64)
    return np.asarray(_dbi_from_stats(S, S2, n), dtype=np.float32)
